# revision 5
# baseline (speedup 1.0000x reference)
"""Trainium2 Bass kernel for nn_AdditiveCouplingLayer.

y = x; y[:, 1::2] += MLP(x[:, 0::2])  with a 512->1024->1024->512 relu MLP.

Strategy: data-parallel over 8 NeuronCores (batch 65536 -> 8192/core),
weights replicated. The MLP's first two layers run in "transposed
activation" space (features on partitions, batch on the free dim) so
every matmul uses the natural weight layout; the host supplies the
masked half of x pre-transposed and pre-cast. Layer 3 swaps the
matmul operand roles (h2 slice stationary, W3 moving) so the
translation comes out in natural [batch, feature] layout — no output
transpose needed.

MODE "fp8" (default): all three matmuls in fp8 e4m3 with DoubleRow
perf mode (2 contraction rows/cycle on the PE — 2x fp16 throughput);
activations are quantized to fp8 by the ACT engine on the fly, PSUM
accumulation stays fp32. Measured rel err ~4.6e-3 (gate 2e-2).
MODE "f16": fp16 matmuls (1 row/cycle), rel err ~3e-5.
"""

import os
import sys

sys.path.insert(0, "/opt/trn_rl_repo")

import numpy as np

B, D, F, H = 65536, 1024, 512, 1024
NCORES = 8
BPC = B // NCORES  # rows per core
TB = 512  # batch tile (matmul free dim)
NBT = BPC // TB  # batch tiles per core
MODE = os.environ.get("BASS_COUPLING_MODE", "fp8")

_cache = {}


def _build(mode):
    import concourse.bacc as bacc
    import concourse.tile as tile
    import concourse.mybir as mybir

    dt = mybir.dt
    AF = mybir.ActivationFunctionType
    fp8 = mode == "fp8"
    adt = dt.float8e4 if fp8 else dt.float16  # activation/weight dtype
    DR = mybir.MatmulPerfMode.DoubleRow if fp8 else None

    nc = bacc.Bacc(
        "TRN2", target_bir_lowering=False, debug=False, num_devices=NCORES
    )

    x_d = nc.dram_tensor("x", [BPC, D], dt.float32, kind="ExternalInput").ap()
    mT_d = nc.dram_tensor("mT", [F, BPC], adt, kind="ExternalInput").ap()
    w_d = {}
    for name, shape in (("w1", [F, H]), ("w2", [H, H]), ("w3", [H, F])):
        w_d[name] = nc.dram_tensor(name, shape, adt, kind="ExternalInput").ap()
    b1_d = nc.dram_tensor("b1m", [128, H // 128], dt.float32, kind="ExternalInput").ap()
    b2_d = nc.dram_tensor("b2m", [128, H // 128], dt.float32, kind="ExternalInput").ap()
    b3r_d = nc.dram_tensor("b3rep", [128, F], dt.float32, kind="ExternalInput").ap()
    y_d = nc.dram_tensor("y", [BPC, D], dt.float32, kind="ExternalOutput").ap()

    with tile.TileContext(nc) as tc:
        with (
            tc.tile_pool(name="wpool", bufs=1) as wpool,
            tc.tile_pool(name="xpool", bufs=3) as xpool,
            tc.tile_pool(name="mpool", bufs=3) as mpool,
            tc.tile_pool(name="hpool", bufs=3) as hpool,
            tc.tile_pool(name="pmm", bufs=6, space="PSUM") as pmm,
        ):
            # --- resident weights/biases ---
            deferred_w = []

            def load_w(name, rows, cols, eng):
                """One 3-dim tile [128, nk, cols] + ONE DMA per weight
                matrix; k-chunk k lives at [:, k, :]. eng=None defers the
                issue (drained from the scalar engine between the first
                L1 evictions)."""
                nk = rows // 128
                big = wpool.tile([128, nk, cols], adt, tag=name, name=name)

                def issue(eng):
                    eng.dma_start(
                        big[:], w_d[name].rearrange("(k p) c -> p k c", p=128)
                    )

                if eng is None:
                    deferred_w.append(issue)
                else:
                    issue(eng)
                return big

            def load_b(name, ap, n):
                # host pre-transposes biases to [128, n/128] so this DMA is
                # contiguous (a "(m p) -> p m" rearrange here is a 4-byte-
                # element gather that takes ~10us and stalls the DMA ring)
                t = wpool.tile([128, n // 128], dt.float32, tag=name)
                nc.scalar.dma_start(t[:], ap[:])
                return t

            # PE warmup: junk matmuls on a zeroed scratch tile keep the PE
            # busy through its HAM activity window while the first real
            # DMAs are in flight, so real matmuls start at 2.4GHz.
            scratch = wpool.tile([128, TB], dt.float16, tag="scratch")
            nc.gpsimd.memset(scratch[:], 0.0)
            pwarm = pmm.tile([128, TB], dt.float32, tag="warm", bufs=1)
            for _ in range(12):
                nc.tensor.matmul(
                    pwarm[:], scratch[:, :128], scratch[:], start=True, stop=True
                )

            # Startup DMA order is the critical path: W1 + tile-0 mT go
            # first on the sync queue; W2/W3 issue from the scalar queue
            # but only AFTER the first L1 evictions (drain_deferred), so
            # they don't steal HBM bandwidth from the W1/mT0 stream.
            # W1 is allocated here but its per-k-chunk DMAs are issued by
            # l1_tile(0) AFTER the tile-0 mT load, so the k-th matmul's
            # operands arrive progressively and the first real matmul can
            # start ~3us earlier than with one monolithic W1 transfer.
            w1t = wpool.tile([128, 4, H], adt, tag="w1")
            b1t = load_b("b1t", b1_d, H)
            b2t = load_b("b2t", b2_d, H)
            b3rep = wpool.tile([128, F], dt.float32, tag="b3rep")
            nc.scalar.dma_start(b3rep[:], b3r_d[:])
            w2t = load_w("w2", H, H, None)
            w3t = load_w("w3", H, F, None)

            def mm_group(psum, pairs, perf_mode=None):
                n = len(pairs)
                for i, (lhsT, rhs) in enumerate(pairs):
                    nc.tensor.matmul(
                        psum[:], lhsT, rhs, start=(i == 0), stop=(i == n - 1),
                        perf_mode=perf_mode,
                    )

            def layer(wt, hin, bt, nout, oname, drain_deferred=False):
                """Transposed-space layer: out [128, nout/128, TB] fp8/fp16
                = relu(W.T@in + b). wt: [128, nk, cols] weight tile; hin:
                [128, nk, TB] activation tile."""
                nk = hin.shape[1]
                out = hpool.tile([128, nout // 128, TB], adt, tag=oname)
                for m in range(nout // 128):
                    p = pmm.tile([128, TB], dt.float32, tag="mm")
                    ms = slice(m * 128, (m + 1) * 128)
                    if fp8:
                        pairs = [
                            (wt[:, 2 * j : 2 * j + 2, ms], hin[:, 2 * j : 2 * j + 2, :])
                            for j in range(nk // 2)
                        ]
                    else:
                        pairs = [
                            (wt[:, k : k + 1, ms], hin[:, k : k + 1, :])
                            for k in range(nk)
                        ]
                    mm_group(p, pairs, perf_mode=DR)
                    nc.scalar.activation(
                        out[:, m : m + 1, :], p[:], AF.Relu, bias=bt[:, m : m + 1]
                    )
                    if drain_deferred and deferred_w:
                        deferred_w.pop(0)(nc.scalar)
                return out

            def l1_tile(bt_i):
                """mT load + layer 1 for one batch tile (issued one tile
                ahead of layers 2/3 so the PE never stalls on the W2/W3
                arrival at startup, and mT is naturally prefetched)."""
                r0 = bt_i * TB
                mT = mpool.tile([128, 4, TB], adt, tag="mbig", name="mbig")
                nc.sync.dma_start(
                    mT[:],
                    mT_d[:, r0 : r0 + TB].rearrange("(j p) c -> p j c", p=128),
                )
                if bt_i == 0:
                    for k in range(4):
                        nc.sync.dma_start(
                            w1t[:, k : k + 1, :],
                            w_d["w1"][k * 128 : (k + 1) * 128, :],
                        )
                return layer(w1t, mT, b1t, H, "h1", drain_deferred=(bt_i == 0))

            h1 = l1_tile(0)
            pending_stores = []
            for bt_i in range(NBT):
                r0 = bt_i * TB

                h1_next = l1_tile(bt_i + 1) if bt_i + 1 < NBT else None

                # y stores ride the scalar HWDGE queue (the sync queue is
                # at ~its single-queue bandwidth limit with the loads),
                # deferred one iteration so the issue never blocks the ACT
                # sequencer waiting on the DVE adds.
                for rows, src in pending_stores:
                    nc.scalar.dma_start(rows, src)
                pending_stores = []

                # x tile (natural layout, needed only for the residual
                # assembly — issued after the mT loads on the same queue).
                # One 3-dim-AP DMA brings all 4 row-chunks side by side.
                xbig = xpool.tile([128, 4 * D], dt.float32, tag="xbig")
                nc.sync.dma_start(
                    xbig[:].rearrange("p (i c) -> p i c", i=4),
                    x_d[r0 : r0 + TB, :].rearrange("(i p) c -> p i c", p=128),
                )
                xb = [xbig[:, i * D : (i + 1) * D] for i in range(4)]
                h2 = layer(w2t, h1, b2t, H, "h2")

                # y is assembled IN PLACE in the x tiles (even columns are
                # already x): odd cols += b3, then += translation.
                for i in range(4):
                    nc.vector.tensor_add(
                        xb[i][:, 1:D:2], xb[i][:, 1:D:2], b3rep[:]
                    )

                # layer 3 in natural layout: stationary = h2 batch-slice,
                # moving = W3 tile  ->  psum[batch128, F]
                for i in range(4):
                    p = pmm.tile([128, F], dt.float32, tag="mm")
                    bs = slice(i * 128, (i + 1) * 128)
                    if fp8:
                        pairs = [
                            (h2[:, 2 * j : 2 * j + 2, bs], w3t[:, 2 * j : 2 * j + 2, :])
                            for j in range(4)
                        ]
                    else:
                        pairs = [
                            (h2[:, k : k + 1, bs], w3t[:, k : k + 1, :])
                            for k in range(8)
                        ]
                    mm_group(p, pairs, perf_mode=DR)
                    rows = y_d[r0 + i * 128 : r0 + (i + 1) * 128, :]
                    if bt_i == NBT - 1:
                        # final tile: split the add+store chain (quarters
                        # for the very last chunk, halves otherwise) and
                        # alternate store queues, so the kernel tail after
                        # the last matmul is as short as possible
                        nsp = 4 if i == 3 else 2
                        w = D // nsp
                        fw = F // nsp
                        for h in range(nsp):
                            osl = slice(h * w + 1, (h + 1) * w, 2)
                            nc.vector.tensor_add(
                                xb[i][:, osl], xb[i][:, osl],
                                p[:, h * fw : (h + 1) * fw],
                            )
                            eng = nc.sync if h % 2 == 0 else nc.scalar
                            eng.dma_start(
                                rows[:, h * w : (h + 1) * w],
                                xb[i][:, h * w : (h + 1) * w],
                            )
                    else:
                        nc.vector.tensor_add(
                            xb[i][:, 1:D:2], xb[i][:, 1:D:2], p[:]
                        )
                        pending_stores.append((rows[:], xb[i][:]))

                if h1_next is not None:
                    h1 = h1_next

    nc.compile()
    return nc


def _get(mode):
    if mode not in _cache:
        _cache[mode] = _build(mode)
    return _cache[mode]


def _in_maps(x, W1, b1, W2, b2, W3, b3):
    import ml_dtypes

    qdt = ml_dtypes.float8_e4m3 if MODE == "fp8" else np.float16

    ws = {
        name: np.asarray(w, np.float32).astype(qdt)
        for name, w in (("w1", W1), ("w2", W2), ("w3", W3))
    }

    common = dict(
        ws,
        b1m=np.ascontiguousarray(np.asarray(b1, np.float32).reshape(-1, 128).T),
        b2m=np.ascontiguousarray(np.asarray(b2, np.float32).reshape(-1, 128).T),
        b3rep=np.ascontiguousarray(
            np.broadcast_to(np.asarray(b3, np.float32), (128, F))
        ),
    )
    x = np.ascontiguousarray(np.asarray(x, np.float32))
    in_maps = []
    for c in range(NCORES):
        xs = x[c * BPC : (c + 1) * BPC]
        masked_t = np.ascontiguousarray(xs[:, 0::2].T)  # [F, BPC] f32
        in_maps.append(dict(common, x=xs, mT=masked_t.astype(qdt)))
    return in_maps


def kernel(x, W1, b1, W2, b2, W3, b3):
    from concourse.bass_utils import run_bass_kernel_spmd

    nc = _get(MODE)
    res = run_bass_kernel_spmd(
        nc, _in_maps(x, W1, b1, W2, b2, W3, b3), core_ids=list(range(NCORES))
    )
    return np.concatenate([res.results[c]["y"] for c in range(NCORES)], axis=0)


# revision 6
# speedup vs baseline: 1.2040x; 1.2040x over previous
"""Trainium2 Bass kernel for nn_AdditiveCouplingLayer — v2: fp8 DoubleRow
matmuls + odd-only device I/O.

y = x; y[:, 1::2] += MLP(x[:, 0::2])  with a 512->1024->1024->512 relu MLP.

Data-parallel over 8 NeuronCores (batch 65536 -> 8192/core), weights
replicated. The even (conditioning) columns of y are exactly x's even
columns, so the device never sees them: the host sends the masked half
pre-transposed+quantized (mT) and the odd columns (xo), the device
returns only yo = xo + b3 + MLP(mT), and the host re-interleaves. This
roughly halves device HBM traffic (2.25MB vs 4.25MB per 512-row tile),
so the kernel is purely PE-bound at the fp8 DoubleRow roofline
(~512k PE cycles/core ~ 218us @2.4GHz).

Matmuls: fp8 e4m3 with DoubleRow perf mode (2 contraction rows/cycle,
2x fp16 throughput), fp32 PSUM accumulation; ACT quantizes the relu
activations to fp8 on the fly. Measured rel err ~4.7e-3 (gate 2e-2).
MODE "f16" keeps the fp16 1-row/cycle path (rel err ~3e-5) with full
x/y device I/O.
"""

import os
import sys

sys.path.insert(0, "/opt/trn_rl_repo")

import numpy as np

B, D, F, H = 65536, 1024, 512, 1024
NCORES = 8
BPC = B // NCORES  # rows per core
TB = 512  # batch tile (matmul free dim)
NBT = BPC // TB  # batch tiles per core
MODE = os.environ.get("BASS_COUPLING_MODE", "fp8")

_cache = {}


def _build(mode):
    import concourse.bacc as bacc
    import concourse.tile as tile
    import concourse.mybir as mybir

    dt = mybir.dt
    AF = mybir.ActivationFunctionType
    fp8 = mode == "fp8"
    adt = dt.float8e4 if fp8 else dt.float16  # activation/weight dtype
    DR = mybir.MatmulPerfMode.DoubleRow if fp8 else None

    nc = bacc.Bacc(
        "TRN2", target_bir_lowering=False, debug=False, num_devices=NCORES
    )

    xo_d = nc.dram_tensor("xo", [BPC, F], dt.float32, kind="ExternalInput").ap()
    mT_d = nc.dram_tensor("mT", [F, BPC], adt, kind="ExternalInput").ap()
    w_d = {}
    for name, shape in (("w1", [F, H]), ("w2", [H, H]), ("w3", [H, F])):
        w_d[name] = nc.dram_tensor(name, shape, adt, kind="ExternalInput").ap()
    b1_d = nc.dram_tensor("b1m", [128, H // 128], dt.float32, kind="ExternalInput").ap()
    b2_d = nc.dram_tensor("b2m", [128, H // 128], dt.float32, kind="ExternalInput").ap()
    b3r_d = nc.dram_tensor("b3rep", [128, F], dt.float32, kind="ExternalInput").ap()
    yo_d = nc.dram_tensor("yo", [BPC, F], dt.float32, kind="ExternalOutput").ap()

    with tile.TileContext(nc) as tc:
        with (
            tc.tile_pool(name="wpool", bufs=1) as wpool,
            tc.tile_pool(name="xpool", bufs=3) as xpool,
            tc.tile_pool(name="mpool", bufs=3) as mpool,
            tc.tile_pool(name="hpool", bufs=3) as hpool,
            tc.tile_pool(name="pmm", bufs=6, space="PSUM") as pmm,
        ):
            # --- resident weights/biases ---
            deferred_w = []

            def load_w(name, rows, cols, eng):
                """One 3-dim tile [128, nk, cols] + ONE DMA per weight
                matrix; k-chunk k lives at [:, k, :]. eng=None defers the
                issue (drained from the scalar engine between the first
                L1 evictions)."""
                nk = rows // 128
                big = wpool.tile([128, nk, cols], adt, tag=name, name=name)

                def issue(eng):
                    eng.dma_start(
                        big[:], w_d[name].rearrange("(k p) c -> p k c", p=128)
                    )

                if eng is None:
                    deferred_w.append(issue)
                else:
                    issue(eng)
                return big

            def load_b(name, ap, n):
                # host pre-transposes biases to [128, n/128] so this DMA is
                # contiguous (a "(m p) -> p m" rearrange here is a 4-byte-
                # element gather that takes ~10us and stalls the DMA ring)
                t = wpool.tile([128, n // 128], dt.float32, tag=name)
                nc.scalar.dma_start(t[:], ap[:])
                return t

            # PE warmup: junk matmuls on a zeroed scratch tile keep the PE
            # busy through its HAM activity window while the first real
            # DMAs are in flight, so real matmuls start at 2.4GHz.
            scratch = wpool.tile([128, TB], dt.float16, tag="scratch")
            nc.gpsimd.memset(scratch[:], 0.0)
            pwarm = pmm.tile([128, TB], dt.float32, tag="warm", bufs=1)
            for _ in range(12):
                nc.tensor.matmul(
                    pwarm[:], scratch[:, :128], scratch[:], start=True, stop=True
                )

            # Startup DMA order is the critical path: W1 + tile-0 mT go
            # first on the sync queue; W2/W3 issue from the scalar queue
            # but only AFTER the first L1 evictions (drain_deferred), so
            # they don't steal HBM bandwidth from the W1/mT0 stream.
            # W1's per-k-chunk DMAs are issued by l1_tile(0) AFTER the
            # tile-0 mT load so the k-th matmul's operands arrive
            # progressively.
            w1t = wpool.tile([128, 4, H], adt, tag="w1")
            b1t = load_b("b1t", b1_d, H)
            b2t = load_b("b2t", b2_d, H)
            b3rep = wpool.tile([128, F], dt.float32, tag="b3rep")
            nc.scalar.dma_start(b3rep[:], b3r_d[:])
            w2t = load_w("w2", H, H, None)
            w3t = load_w("w3", H, F, None)

            def mm_group(psum, pairs, perf_mode=None):
                n = len(pairs)
                for i, (lhsT, rhs) in enumerate(pairs):
                    nc.tensor.matmul(
                        psum[:], lhsT, rhs, start=(i == 0), stop=(i == n - 1),
                        perf_mode=perf_mode,
                    )

            def layer(wt, hin, bt, nout, oname, drain_deferred=False):
                """Transposed-space layer: out [128, nout/128, TB] fp8/fp16
                = relu(W.T@in + b). wt: [128, nk, cols] weight tile; hin:
                [128, nk, TB] activation tile."""
                nk = hin.shape[1]
                out = hpool.tile([128, nout // 128, TB], adt, tag=oname)
                for m in range(nout // 128):
                    p = pmm.tile([128, TB], dt.float32, tag="mm")
                    ms = slice(m * 128, (m + 1) * 128)
                    if fp8:
                        pairs = [
                            (wt[:, 2 * j : 2 * j + 2, ms], hin[:, 2 * j : 2 * j + 2, :])
                            for j in range(nk // 2)
                        ]
                    else:
                        pairs = [
                            (wt[:, k : k + 1, ms], hin[:, k : k + 1, :])
                            for k in range(nk)
                        ]
                    mm_group(p, pairs, perf_mode=DR)
                    nc.scalar.activation(
                        out[:, m : m + 1, :], p[:], AF.Relu, bias=bt[:, m : m + 1]
                    )
                    if drain_deferred and deferred_w:
                        deferred_w.pop(0)(nc.scalar)
                return out

            def l1_tile(bt_i):
                """mT load + layer 1 for one batch tile (issued one tile
                ahead of layers 2/3 so the PE never stalls on the W2/W3
                arrival at startup, and mT is naturally prefetched)."""
                r0 = bt_i * TB
                mT = mpool.tile([128, 4, TB], adt, tag="mbig", name="mbig")
                nc.sync.dma_start(
                    mT[:],
                    mT_d[:, r0 : r0 + TB].rearrange("(j p) c -> p j c", p=128),
                )
                if bt_i == 0:
                    for k in range(4):
                        nc.sync.dma_start(
                            w1t[:, k : k + 1, :],
                            w_d["w1"][k * 128 : (k + 1) * 128, :],
                        )
                return layer(w1t, mT, b1t, H, "h1", drain_deferred=(bt_i == 0))

            h1 = l1_tile(0)
            pending_stores = []
            for bt_i in range(NBT):
                r0 = bt_i * TB

                h1_next = l1_tile(bt_i + 1) if bt_i + 1 < NBT else None

                # yo stores ride the scalar HWDGE queue, deferred one
                # iteration so the issue never blocks the ACT sequencer
                # waiting on the DVE adds.
                for rows, src in pending_stores:
                    nc.scalar.dma_start(rows, src)
                pending_stores = []

                # xo tile (odd columns, natural layout — the residual).
                # One 3-dim-AP DMA brings all 4 row-chunks side by side.
                xobig = xpool.tile([128, 4, F], dt.float32, tag="xobig")
                nc.sync.dma_start(
                    xobig[:],
                    xo_d[r0 : r0 + TB, :].rearrange("(i p) c -> p i c", p=128),
                )
                h2 = layer(w2t, h1, b2t, H, "h2")

                # yo = xo + b3 + translation; b3 pre-added while L3 runs.
                for i in range(4):
                    nc.vector.tensor_add(
                        xobig[:, i : i + 1, :], xobig[:, i : i + 1, :], b3rep[:]
                    )

                # layer 3 in natural layout: stationary = h2 batch-slice,
                # moving = W3 tile  ->  psum[batch128, F]
                for i in range(4):
                    p = pmm.tile([128, F], dt.float32, tag="mm")
                    bs = slice(i * 128, (i + 1) * 128)
                    if fp8:
                        pairs = [
                            (h2[:, 2 * j : 2 * j + 2, bs], w3t[:, 2 * j : 2 * j + 2, :])
                            for j in range(4)
                        ]
                    else:
                        pairs = [
                            (h2[:, k : k + 1, bs], w3t[:, k : k + 1, :])
                            for k in range(8)
                        ]
                    mm_group(p, pairs, perf_mode=DR)
                    rows = yo_d[r0 + i * 128 : r0 + (i + 1) * 128, :]
                    xoi = xobig[:, i : i + 1, :]
                    if bt_i == NBT - 1:
                        # final tile: split the add+store chain and
                        # alternate store queues so the kernel tail after
                        # the last matmul is as short as possible
                        nsp = 4 if i == 3 else 2
                        fw = F // nsp
                        for h in range(nsp):
                            cs = slice(h * fw, (h + 1) * fw)
                            nc.vector.tensor_add(
                                xobig[:, i : i + 1, cs],
                                xobig[:, i : i + 1, cs],
                                p[:, cs],
                            )
                            eng = nc.sync if h % 2 == 0 else nc.scalar
                            eng.dma_start(rows[:, cs], xobig[:, i : i + 1, cs])
                    else:
                        nc.vector.tensor_add(xoi, xoi, p[:])
                        pending_stores.append((rows[:], xoi))

                if h1_next is not None:
                    h1 = h1_next

    nc.compile()
    return nc


def _get(mode):
    if mode not in _cache:
        _cache[mode] = _build(mode)
    return _cache[mode]


def _in_maps(x, W1, b1, W2, b2, W3, b3):
    import ml_dtypes

    qdt = ml_dtypes.float8_e4m3 if MODE == "fp8" else np.float16

    ws = {
        name: np.asarray(w, np.float32).astype(qdt)
        for name, w in (("w1", W1), ("w2", W2), ("w3", W3))
    }

    common = dict(
        ws,
        b1m=np.ascontiguousarray(np.asarray(b1, np.float32).reshape(-1, 128).T),
        b2m=np.ascontiguousarray(np.asarray(b2, np.float32).reshape(-1, 128).T),
        b3rep=np.ascontiguousarray(
            np.broadcast_to(np.asarray(b3, np.float32), (128, F))
        ),
    )
    x = np.asarray(x, np.float32)
    in_maps = []
    for c in range(NCORES):
        xs = x[c * BPC : (c + 1) * BPC]
        masked_t = np.ascontiguousarray(xs[:, 0::2].T)  # [F, BPC] f32
        in_maps.append(
            dict(
                common,
                xo=np.ascontiguousarray(xs[:, 1::2]),
                mT=masked_t.astype(qdt),
            )
        )
    return in_maps


def kernel(x, W1, b1, W2, b2, W3, b3):
    from concourse.bass_utils import run_bass_kernel_spmd

    nc = _get(MODE)
    x = np.asarray(x, np.float32)
    res = run_bass_kernel_spmd(
        nc, _in_maps(x, W1, b1, W2, b2, W3, b3), core_ids=list(range(NCORES))
    )
    y = np.empty((B, D), dtype=np.float32)
    y[:, 0::2] = x[:, 0::2]
    yo = np.concatenate([res.results[c]["yo"] for c in range(NCORES)], axis=0)
    y[:, 1::2] = yo
    return y
